# revision 1
# baseline (speedup 1.0000x reference)
# MoE EnhancedGatedFusion kernel for 8x TRN2 NeuronCores (expert-parallel).
#
# Decomposition (validated in proto.py, rel err ~3e-7 vs reference):
#   host : router logits -> top2 -> softmax gates -> dispatch by expert
#   L1   : per-core (expert e): H_T = gate * silu(We[e].T-contract @ XT + be[e])
#          layout [d_out, tokens] so downstream needs no transposes anywhere.
#   host : column-gather H_T into per-core AT/BT (slot1/slot2 aligned to
#          token order); pure data movement, no FLOPs.
#   L2   : per-core (1024 tokens): combT = AT+BT; out = combT.T @ Wo (+bo via
#          host-folded XIN = x_shard + bo); y = XIN + out; RMS-norm * norm_w.
#
# Matmuls run as float32r (TF32-like): 1 cycle/row (same as bf16) vs 4 for
# fp32, with f32 storage and fp32 PSUM accumulation.
import sys
import types

sys.path.insert(0, "/opt/trn_rl_repo")

import numpy as np


def _install_ntff_hook():
    # antenv.axon_hooks is missing in this image; shim it so
    # run_bass_kernel_spmd(trace=True) can drive NTFF profiling.
    if "antenv.axon_hooks" in sys.modules:
        return
    try:
        from trn_agent_boot.trn_boot import _ntff_profile_via_ctypes

        hook = _ntff_profile_via_ctypes("/opt/axon/libaxon_pjrt.so")
    except Exception:
        hook = None
    mod = types.ModuleType("antenv.axon_hooks")
    mod.get_axon_ntff_profile_hook = lambda: hook
    mod.set_axon_ntff_profile_hook = lambda h: None
    sys.modules["antenv.axon_hooks"] = mod


_install_ntff_hook()

import concourse.bacc as bacc
import concourse.bass as bass
import concourse.tile as tile
from concourse import mybir
from concourse.bass_utils import run_bass_kernel_spmd

F32 = mybir.dt.float32
F32R = mybir.dt.float32r
P = 128
NCORE = 8


def _chunks(total, size):
    out = []
    o = 0
    while o < total:
        out.append((o, min(size, total - o)))
        o += size
    return out


def build_l1(D, Bcap):
    """Per-core expert FFN: H[d_out, n] = g[n] * silu(sum_k W[k,d_out]*XT[k,n] + be[d_out]).

    XT_T is host-pretiled [C, P, K, 512] (zero-padded ragged tail) and W_T is
    [F, P, K, P] so every DMA reads long contiguous per-partition runs.
    """
    K = D // P          # k-tiles
    F = D // P          # feat (d_out) tiles
    chunks = _chunks(Bcap, 512)
    C = len(chunks)
    nc = bacc.Bacc("TRN2", target_bir_lowering=False, debug=False)
    XT = nc.dram_tensor("XT", [C, P, K, 512], F32R, kind="ExternalInput")
    W = nc.dram_tensor("W", [F, P, K, P], F32R, kind="ExternalInput")
    BE = nc.dram_tensor("BE", [D], F32, kind="ExternalInput")
    G = nc.dram_tensor("G", [Bcap], F32, kind="ExternalInput")
    H = nc.dram_tensor("H", [D, Bcap], F32, kind="ExternalOutput")

    Hr = H[:, :].rearrange("(fo p) n -> p fo n", p=P)

    FH = 2               # W resident in halves: two passes over feat tiles
    FPH = F // FH
    with tile.TileContext(nc) as tc:
        with (
            tc.tile_pool(name="consts", bufs=1) as consts,
            tc.tile_pool(name="xt", bufs=3) as xtp,
            tc.tile_pool(name="wf", bufs=1) as wfp,
            tc.tile_pool(name="hout", bufs=4) as hp,
            tc.tile_pool(name="ps", bufs=4, space="PSUM") as psp,
        ):
            g_sb = consts.tile([P, Bcap], F32)
            gap = G[:]
            g_bcast = bass.AP(tensor=gap.tensor, offset=gap.offset, ap=[[0, P]] + list(gap.ap))
            nc.sync.dma_start(g_sb[:], g_bcast)
            be_sb = consts.tile([P, F], F32)
            nc.sync.dma_start(be_sb[:], BE[:].rearrange("(f p) -> p f", p=P))

            for fh in range(FH):
                w_tiles = [None] * FPH
                for ci, (c0, cn) in enumerate(chunks):
                    xt_c = xtp.tile([P, K, 512], F32R, tag="xt", name="xt")
                    nc.sync.dma_start(xt_c[:], XT[ci])
                    if ci == 0:
                        # W half issued right after the first token chunk so
                        # the f-loop streams while later chunks load.
                        for j in range(FPH):
                            f = fh * FPH + j
                            w_f = wfp.tile([P, K, P], F32R, tag=f"wf{j}", name=f"wf{j}")
                            nc.sync.dma_start(w_f[:], W[f])
                            w_tiles[j] = w_f
                    for j in range(FPH):
                        f = fh * FPH + j
                        ps = psp.tile([P, 512], F32, tag="ps", name="ps")
                        for k in range(K):
                            nc.tensor.matmul(
                                ps[:, :cn],
                                lhsT=w_tiles[j][:, k, :],
                                rhs=xt_c[:, k, :cn],
                                start=(k == 0),
                                stop=(k == K - 1),
                            )
                        h_t = hp.tile([P, 512], F32, tag="h", name="h")
                        nc.scalar.activation(
                            h_t[:, :cn],
                            ps[:, :cn],
                            mybir.ActivationFunctionType.Silu,
                            bias=be_sb[:, f : f + 1],
                            scale=1.0,
                        )
                        nc.vector.tensor_mul(h_t[:, :cn], h_t[:, :cn], g_sb[:, c0 : c0 + cn])
                        nc.sync.dma_start(Hr[:, f, c0 : c0 + cn], h_t[:, :cn])
    nc.compile()
    return nc


def build_l2(D, TPC, eps=1e-6):
    """Per-core combine + output proj + residual + RMS norm.

    Y[t, j] = nw[j] * (XIN[t,j] + sum_k (AT+BT)[k,t]*Wo[k,j]) / rms(t)
    XIN = x_shard + bo (bo folded on host).
    """
    K = D // P
    M = TPC // P
    nc = bacc.Bacc("TRN2", target_bir_lowering=False, debug=False)
    AT = nc.dram_tensor("AT", [D, TPC], F32R, kind="ExternalInput")
    BT = nc.dram_tensor("BT", [D, TPC], F32R, kind="ExternalInput")
    XIN = nc.dram_tensor("XIN", [TPC, D], F32, kind="ExternalInput")
    WO = nc.dram_tensor("WO", [D, D], F32R, kind="ExternalInput")
    NW = nc.dram_tensor("NW", [D], F32, kind="ExternalInput")
    Y = nc.dram_tensor("Y", [TPC, D], F32, kind="ExternalOutput")

    ATr = AT[:, :].rearrange("(ko p) n -> p ko n", p=P)
    BTr = BT[:, :].rearrange("(ko p) n -> p ko n", p=P)
    WOr = WO[:, :].rearrange("(ko p) f -> p ko f", p=P)

    with tile.TileContext(nc) as tc:
        with (
            tc.tile_pool(name="consts", bufs=1) as consts,
            tc.tile_pool(name="comb", bufs=1) as combp,
            tc.tile_pool(name="btp", bufs=2) as btp,
            tc.tile_pool(name="wo", bufs=12) as wop,
            tc.tile_pool(name="yall", bufs=1) as yallp,
            tc.tile_pool(name="sq", bufs=3) as sqp,
            tc.tile_pool(name="yn", bufs=2) as ynp,
            tc.tile_pool(name="ssm", bufs=1) as ssmp,
            tc.tile_pool(name="stat", bufs=4) as statp,
            tc.tile_pool(name="ps", bufs=1, space="PSUM") as psp,
        ):
            nw_sb = consts.tile([P, D], F32)
            nwap = NW[:]
            nw_bcast = bass.AP(
                tensor=nwap.tensor, offset=nwap.offset, ap=[[0, P]] + list(nwap.ap)
            )
            nc.sync.dma_start(nw_sb[:], nw_bcast)
            eps_sb = consts.tile([P, 1], F32)
            nc.vector.memset(eps_sb[:], eps)

            NC4 = D // 512
            # comb_k triplets (AT_k, BT_k, add) interleaved with n0's wo
            # k-slices: the first matmuls start ~1MB into the load instead of
            # behind the whole 16.8MB combine assembly.
            comb_ks = []
            wo_n0 = []
            for k in range(K):
                c_k = combp.tile([P, TPC], F32R, tag=f"cb{k}", name=f"cb{k}")
                nc.sync.dma_start(c_k[:], ATr[:, k, :])
                bt_k = btp.tile([P, TPC], F32R, tag="bt", name="bt")
                nc.sync.dma_start(bt_k[:], BTr[:, k, :])
                nc.vector.tensor_add(c_k[:], c_k[:], bt_k[:])
                comb_ks.append(c_k)
                w0k = wop.tile([P, 512], F32R, tag="wo", name="wo")
                nc.sync.dma_start(w0k[:], WOr[:, k, 0:512])
                wo_n0.append(w0k)
            # y_all preloaded with the residual input (XIN = x + bo): psum
            # evictions then just add into it.
            y_all = yallp.tile([P, M, D], F32)
            nc.sync.dma_start(y_all[:], XIN[:, :].rearrange("(m p) d -> p m d", p=P))

            ss_m = [ssmp.tile([P, 1], F32, tag=f"ssm{m}", name=f"ssm{m}")
                    for m in range(M)]

            for n in range(NC4):
                n0 = n * 512
                if n == 0:
                    wo_k = wo_n0
                else:
                    wo_k = []
                    for k in range(K):
                        wk = wop.tile([P, 512], F32R, tag="wo", name="wo")
                        nc.sync.dma_start(wk[:], WOr[:, k, n0 : n0 + 512])
                        wo_k.append(wk)
                pss = [psp.tile([P, 512], F32, tag=f"ps{m}", name=f"ps{m}")
                       for m in range(M)]
                for k in range(K):
                    for m in range(M):
                        nc.tensor.matmul(
                            pss[m][:],
                            lhsT=comb_ks[k][:, m * P : (m + 1) * P],
                            rhs=wo_k[k][:],
                            start=(k == 0),
                            stop=(k == K - 1),
                        )
                for m in range(M):
                    y_slice = y_all[:, m, n0 : n0 + 512]
                    nc.vector.tensor_add(y_slice, y_slice, pss[m][:])
                    # incremental RMS stats: ss_m[m] += sum(y_slice^2)
                    sq = sqp.tile([P, 512], F32, tag="sq", name="sq")
                    ssp = statp.tile([P, 1], F32, tag="ssp", name="ssp")
                    nc.scalar.activation(
                        sq[:],
                        y_slice,
                        mybir.ActivationFunctionType.Square,
                        accum_out=ssp[:],
                    )
                    if n == 0:
                        nc.vector.tensor_copy(ss_m[m][:], ssp[:])
                    else:
                        nc.vector.tensor_add(ss_m[m][:], ss_m[m][:], ssp[:])
            for m in range(M):
                y_m = y_all[:, m, :]
                rms = statp.tile([P, 1], F32, tag="rms", name="rms")
                nc.scalar.activation(
                    rms[:],
                    ss_m[m][:],
                    mybir.ActivationFunctionType.Sqrt,
                    bias=eps_sb[:],
                    scale=1.0 / D,
                )
                rstd = statp.tile([P, 1], F32, tag="rstd", name="rstd")
                nc.vector.reciprocal(rstd[:], rms[:])
                yn = ynp.tile([P, D], F32, tag="yn", name="yn")
                nc.scalar.activation(
                    yn[:],
                    y_m,
                    mybir.ActivationFunctionType.Identity,
                    bias=0.0,
                    scale=rstd[:],
                )
                nc.vector.tensor_mul(yn[:], yn[:], nw_sb[:])
                nc.sync.dma_start(Y[m * P : (m + 1) * P, :], yn[:])
    nc.compile()
    return nc


def host_dispatch(xf, Wr, br):
    """Router + top-2 + softmax gates + expert grouping. Returns dispatch info."""
    T, D = xf.shape
    E = Wr.shape[1]
    logits = xf @ Wr + br
    i1 = np.argmax(logits, axis=1)
    l2 = logits.copy()
    l2[np.arange(T), i1] = -np.inf
    i2 = np.argmax(l2, axis=1)
    v1 = logits[np.arange(T), i1]
    v2 = logits[np.arange(T), i2]
    e2 = np.exp(v2 - v1)
    g1 = (1.0 / (1.0 + e2)).astype(np.float32)
    g2 = (e2 / (1.0 + e2)).astype(np.float32)

    # flat slots (t,s) grouped by expert, stable by (token, slot)
    ee = np.stack([i1, i2], 1).ravel()          # [2T]
    gg = np.stack([g1, g2], 1).ravel()
    tt = np.repeat(np.arange(T), 2)
    order = np.argsort(ee, kind="stable")
    counts = np.bincount(ee, minlength=E)
    starts = np.concatenate([[0], np.cumsum(counts)[:-1]])
    rank = np.empty(2 * T, np.int64)
    rank[order] = np.arange(2 * T)
    pos = rank - starts[ee]                      # position within expert's list
    return dict(
        e1=i1, e2=i2, counts=counts, order=order, starts=starts,
        pos=pos.reshape(T, 2), tok=tt, gate=gg,
    )


def run_moe(x, Wr, br, We, be, Wo, bo, norm_w, trace=False, tmpdir=None):
    B, S, D = x.shape
    E = We.shape[0]
    T = B * S
    TPC = T // NCORE
    xf = np.ascontiguousarray(x.reshape(T, D).astype(np.float32))
    d = host_dispatch(xf, np.asarray(Wr, np.float32), np.asarray(br, np.float32))
    counts = d["counts"]
    Bcap = int(np.ceil(max(counts.max(), 256) / 256) * 256)

    # --- L1 inputs ---
    We = np.asarray(We, np.float32)
    be = np.asarray(be, np.float32)
    K = D // P
    F = D // P
    C = (Bcap + 511) // 512
    Bpad = C * 512
    in1 = []
    for e in range(E):
        sel = d["order"][d["starts"][e] : d["starts"][e] + counts[e]]
        Xg = np.zeros((Bpad, D), np.float32)
        Xg[: counts[e]] = xf[d["tok"][sel]]
        # [C, P, K, 512]: contiguous 32KB per (chunk, partition) DMA runs
        XT_T = np.ascontiguousarray(
            Xg.reshape(C, 512, K, P).transpose(0, 3, 2, 1)
        )
        W_T = np.ascontiguousarray(
            We[e].reshape(K, P, F, P).transpose(2, 1, 0, 3)
        )
        G = np.zeros((Bcap,), np.float32)
        G[: counts[e]] = d["gate"][sel]
        in1.append({"XT": XT_T, "W": W_T, "BE": be[e], "G": G})

    nc1 = build_l1(D, Bcap)
    r1 = run_bass_kernel_spmd(
        nc1, in1, list(range(NCORE)), trace=trace,
        tmpdir=(tmpdir + "/l1" if tmpdir else None),
    )
    H = np.stack([r1.results[e]["H"] for e in range(E)])  # [E, D, Bcap]

    # --- gather AT/BT per core (column gathers, no transposes) ---
    Wo = np.asarray(Wo, np.float32)
    bo = np.asarray(bo, np.float32)
    nw = np.asarray(norm_w, np.float32)
    e1, e2, pos = d["e1"], d["e2"], d["pos"]
    in2 = []
    for c in range(NCORE):
        tl = np.arange(c * TPC, (c + 1) * TPC)
        AT = np.empty((D, TPC), np.float32)
        BT = np.empty((D, TPC), np.float32)
        for e in range(E):
            s1 = e1[tl] == e
            if s1.any():
                AT[:, s1] = H[e][:, pos[tl[s1], 0]]
            s2 = e2[tl] == e
            if s2.any():
                BT[:, s2] = H[e][:, pos[tl[s2], 1]]
        XIN = xf[tl] + bo[None, :]
        in2.append({"AT": AT, "BT": BT, "XIN": XIN, "WO": Wo, "NW": nw})

    nc2 = build_l2(D, TPC)
    r2 = run_bass_kernel_spmd(
        nc2, in2, list(range(NCORE)), trace=trace,
        tmpdir=(tmpdir + "/l2" if tmpdir else None),
    )
    Y = np.concatenate([r2.results[c]["Y"] for c in range(NCORE)], axis=0)
    times = (r1.exec_time_ns, r2.exec_time_ns)
    return Y.reshape(B, S, D).astype(x.dtype), times


# ----------------------------------------------------------------------------
# Harness entry point: full (unsharded) inputs -> full output.
# ----------------------------------------------------------------------------
_L1_CACHE = {}
_L2_CACHE = {}


def kernel(x, Wr, br, We, be, Wo, bo, norm_w):
    B, S, D = x.shape
    E = We.shape[0]
    T = B * S
    TPC = T // NCORE
    K = D // P
    F = D // P
    xf = np.ascontiguousarray(np.asarray(x, np.float32).reshape(T, D))
    d = host_dispatch(xf, np.asarray(Wr, np.float32), np.asarray(br, np.float32))
    counts = d["counts"]
    Bcap = int(np.ceil(max(int(counts.max()), 256) / 256) * 256)
    C = (Bcap + 511) // 512
    Bpad = C * 512

    We_f = np.asarray(We, np.float32)
    be_f = np.asarray(be, np.float32)
    in1 = []
    for e in range(E):
        sel = d["order"][d["starts"][e] : d["starts"][e] + counts[e]]
        Xg = np.zeros((Bpad, D), np.float32)
        Xg[: counts[e]] = xf[d["tok"][sel]]
        XT_T = np.ascontiguousarray(Xg.reshape(C, 512, K, P).transpose(0, 3, 2, 1))
        W_T = np.ascontiguousarray(We_f[e].reshape(K, P, F, P).transpose(2, 1, 0, 3))
        G = np.zeros((Bcap,), np.float32)
        G[: counts[e]] = d["gate"][sel]
        in1.append({"XT": XT_T, "W": W_T, "BE": be_f[e], "G": G})

    if (D, Bcap) not in _L1_CACHE:
        _L1_CACHE[(D, Bcap)] = build_l1(D, Bcap)
    r1 = run_bass_kernel_spmd(_L1_CACHE[(D, Bcap)], in1, list(range(NCORE)))
    H = np.stack([r1.results[e]["H"] for e in range(E)])

    Wo_f = np.asarray(Wo, np.float32)
    bo_f = np.asarray(bo, np.float32)
    nw_f = np.asarray(norm_w, np.float32)
    e1, e2, pos = d["e1"], d["e2"], d["pos"]
    in2 = []
    for c in range(NCORE):
        tl = np.arange(c * TPC, (c + 1) * TPC)
        AT = np.empty((D, TPC), np.float32)
        BT = np.empty((D, TPC), np.float32)
        for e in range(E):
            s1 = e1[tl] == e
            if s1.any():
                AT[:, s1] = H[e][:, pos[tl[s1], 0]]
            s2 = e2[tl] == e
            if s2.any():
                BT[:, s2] = H[e][:, pos[tl[s2], 1]]
        XIN = xf[tl] + bo_f[None, :]
        in2.append({"AT": AT, "BT": BT, "XIN": XIN, "WO": Wo_f, "NW": nw_f})

    if (D, TPC) not in _L2_CACHE:
        _L2_CACHE[(D, TPC)] = build_l2(D, TPC)
    r2 = run_bass_kernel_spmd(_L2_CACHE[(D, TPC)], in2, list(range(NCORE)))
    Y = np.concatenate([r2.results[c]["Y"] for c in range(NCORE)], axis=0)
    return Y.reshape(B, S, D).astype(np.asarray(x).dtype)



# revision 3
# speedup vs baseline: 1.1926x; 1.1926x over previous
# MoE EnhancedGatedFusion kernel for 8x TRN2 NeuronCores (expert-parallel).
#
# Decomposition:
#   host : router logits -> top2 -> softmax gates -> dispatch by expert
#   L1   : per-core (expert e): H_T[d_out, n] = silu(We[e].T-contract @ XT + be[e])
#          (ungated; bf16 matmul operands, fp32 PSUM + fp32 H output)
#   host : combine CT = g1*A + g2*B during the token un-shuffle (the
#          "all-to-all return" glue), downcast once to bf16.
#   L2   : per-core (1024 tokens): out = CT.T @ Wo; y = XIN + out (XIN =
#          x_shard + bo folded on host, fp32); RMS-norm * norm_w.
#
# Matmul operands are bf16: same 1 cycle/row PE rate as fp32r but half the
# HBM/SBUF traffic, and FWL (fast weight load) kicks in for non-fp32 dtypes
# so LDWEIGHTS hides under the 64-deep PE reorder window.
import sys
import types

sys.path.insert(0, "/opt/trn_rl_repo")

import numpy as np
import ml_dtypes

BF16 = np.dtype(ml_dtypes.bfloat16)


def _install_ntff_hook():
    # antenv.axon_hooks is missing in this image; shim it so
    # run_bass_kernel_spmd(trace=True) can drive NTFF profiling.
    if "antenv.axon_hooks" in sys.modules:
        return
    try:
        from trn_agent_boot.trn_boot import _ntff_profile_via_ctypes

        hook = _ntff_profile_via_ctypes("/opt/axon/libaxon_pjrt.so")
    except Exception:
        hook = None
    mod = types.ModuleType("antenv.axon_hooks")
    mod.get_axon_ntff_profile_hook = lambda: hook
    mod.set_axon_ntff_profile_hook = lambda h: None
    sys.modules["antenv.axon_hooks"] = mod


_install_ntff_hook()

import concourse.bacc as bacc
import concourse.bass as bass
import concourse.tile as tile
from concourse import mybir
from concourse.bass_utils import run_bass_kernel_spmd

F32 = mybir.dt.float32
BF = mybir.dt.bfloat16
P = 128
NCORE = 8


def _chunks(total, size):
    out = []
    o = 0
    while o < total:
        out.append((o, min(size, total - o)))
        o += size
    return out


def build_l1(D, Bcap):
    """Per-core expert FFN: H[d_out, n] = silu(sum_k W[k,d_out]*XT[k,n] + be[d_out]).

    XT_T is host-pretiled bf16 [C, P, K, 512] (zero-padded ragged tail) and
    W_T is bf16 [F, P, K, P]; W is fully SBUF-resident (8.4MB), XT streams
    through once.  H output is fp32, staged per chunk in SBUF and written
    with a single DMA per chunk.
    """
    K = D // P          # k-tiles
    F = D // P          # feat (d_out) tiles
    chunks = _chunks(Bcap, 512)
    C = len(chunks)
    nc = bacc.Bacc("TRN2", target_bir_lowering=False, debug=False)
    XT = nc.dram_tensor("XT", [C, P, K, 512], BF, kind="ExternalInput")
    W = nc.dram_tensor("W", [F, P, K, P], BF, kind="ExternalInput")
    BE = nc.dram_tensor("BE", [D], F32, kind="ExternalInput")
    H = nc.dram_tensor("H", [D, Bcap], F32, kind="ExternalOutput")

    Hr = H[:, :].rearrange("(fo p) n -> p fo n", p=P)

    with tile.TileContext(nc) as tc:
        with (
            tc.tile_pool(name="consts", bufs=1) as consts,
            tc.tile_pool(name="xt", bufs=2) as xtp,
            tc.tile_pool(name="wf", bufs=1) as wfp,
            tc.tile_pool(name="hout", bufs=2) as hp,
            tc.tile_pool(name="ps", bufs=4, space="PSUM") as psp,
        ):
            be_sb = consts.tile([P, F], F32)
            nc.sync.dma_start(be_sb[:], BE[:].rearrange("(f p) -> p f", p=P))

            w_tiles = [None] * F
            for ci, (c0, cn) in enumerate(chunks):
                xt_c = xtp.tile([P, K, 512], BF, tag="xt", name="xt")
                if ci == 0:
                    # split first chunk's load so matmuls start after the
                    # first k-slices land instead of the whole 2MB.
                    for kq in range(4):
                        nc.sync.dma_start(
                            xt_c[:, kq * 4 : (kq + 1) * 4, :cn],
                            XT[ci, :, kq * 4 : (kq + 1) * 4, :cn],
                        )
                    for f in range(F):
                        w_f = wfp.tile([P, K, P], BF, tag=f"wf{f}", name=f"wf{f}")
                        nc.sync.dma_start(w_f[:], W[f])
                        w_tiles[f] = w_f
                else:
                    nc.sync.dma_start(xt_c[:, :, :cn], XT[ci, :, :, :cn])
                h_c = hp.tile([P, F, 512], F32, tag="h", name="h")
                for f in range(F):
                    ps = psp.tile([P, 512], F32, tag="ps", name="ps")
                    for k in range(K):
                        nc.tensor.matmul(
                            ps[:, :cn],
                            lhsT=w_tiles[f][:, k, :],
                            rhs=xt_c[:, k, :cn],
                            start=(k == 0),
                            stop=(k == K - 1),
                        )
                    nc.scalar.activation(
                        h_c[:, f, :cn],
                        ps[:, :cn],
                        mybir.ActivationFunctionType.Silu,
                        bias=be_sb[:, f : f + 1],
                        scale=1.0,
                    )
                nc.sync.dma_start(Hr[:, :, c0 : c0 + cn], h_c[:, :, :cn])
    nc.compile()
    return nc


def build_l2(D, TPC, eps=1e-6):
    """Per-core combine + output proj + residual + RMS norm.

    Y[t, j] = nw[j] * (XIN[t,j] + sum_k CT[k,t]*Wo[k,j]) / rms(t)
    CT = g1*A + g2*B (host-combined, bf16); XIN = x_shard + bo (fp32).
    """
    K = D // P
    M = TPC // P
    NC4 = D // 512
    nc = bacc.Bacc("TRN2", target_bir_lowering=False, debug=False)
    CT = nc.dram_tensor("CT", [D, TPC], BF, kind="ExternalInput")
    XIN = nc.dram_tensor("XIN", [TPC, D], F32, kind="ExternalInput")
    WO = nc.dram_tensor("WO", [D, D], BF, kind="ExternalInput")
    NW = nc.dram_tensor("NW", [D], F32, kind="ExternalInput")
    Y = nc.dram_tensor("Y", [TPC, D], F32, kind="ExternalOutput")

    CTr = CT[:, :].rearrange("(ko p) n -> p ko n", p=P)
    WOr = WO[:, :].rearrange("(ko p) f -> p ko f", p=P)
    XINr = XIN[:, :].rearrange("(m p) d -> p m d", p=P)

    with tile.TileContext(nc) as tc:
        with (
            tc.tile_pool(name="consts", bufs=1) as consts,
            tc.tile_pool(name="ct", bufs=1) as ctp,
            tc.tile_pool(name="wo", bufs=1) as wop,
            tc.tile_pool(name="yall", bufs=1) as yallp,
            tc.tile_pool(name="sq", bufs=3) as sqp,
            tc.tile_pool(name="yn", bufs=2) as ynp,
            tc.tile_pool(name="ssm", bufs=1) as ssmp,
            tc.tile_pool(name="stat", bufs=4) as statp,
            tc.tile_pool(name="ps", bufs=1, space="PSUM") as psp,
        ):
            nw_sb = consts.tile([P, D], F32)
            nwap = NW[:]
            nw_bcast = bass.AP(
                tensor=nwap.tensor, offset=nwap.offset, ap=[[0, P]] + list(nwap.ap)
            )
            nc.sync.dma_start(nw_sb[:], nw_bcast)
            eps_sb = consts.tile([P, 1], F32)
            nc.vector.memset(eps_sb[:], eps)

            # CT k-slices first (small, unblock the k-loop), then wo n0
            # slices, then XIN per-m (needed only at first psum eviction).
            ct_ks = []
            for k in range(K):
                c_k = ctp.tile([P, TPC], BF, tag=f"ct{k}", name=f"ct{k}")
                nc.sync.dma_start(c_k[:], CTr[:, k, :])
                ct_ks.append(c_k)
            wo_tiles = {}
            for k in range(K):
                w0k = wop.tile([P, 512], BF, tag=f"wo{k}n0", name=f"wo{k}n0")
                nc.sync.dma_start(w0k[:], WOr[:, k, 0:512])
                wo_tiles[(k, 0)] = w0k
            # y_all preloaded with the residual input (XIN = x + bo): psum
            # evictions then just add into it.  Per-m loads so the first
            # eviction doesn't wait for the full 8.4MB.
            y_all = yallp.tile([P, M, D], F32)
            for m in range(M):
                nc.sync.dma_start(y_all[:, m, :], XINr[:, m, :])

            ss_m = [ssmp.tile([P, 1], F32, tag=f"ssm{m}", name=f"ssm{m}")
                    for m in range(M)]

            for n in range(NC4):
                n0 = n * 512
                if n + 1 < NC4:
                    for k in range(K):
                        wk = wop.tile([P, 512], BF, tag=f"wo{k}n{n+1}",
                                      name=f"wo{k}n{n+1}")
                        nc.sync.dma_start(wk[:], WOr[:, k, n0 + 512 : n0 + 1024])
                        wo_tiles[(k, n + 1)] = wk
                pss = [psp.tile([P, 512], F32, tag=f"ps{m}", name=f"ps{m}")
                       for m in range(M)]
                for k in range(K):
                    for m in range(M):
                        nc.tensor.matmul(
                            pss[m][:],
                            lhsT=ct_ks[k][:, m * P : (m + 1) * P],
                            rhs=wo_tiles[(k, n)][:],
                            start=(k == 0),
                            stop=(k == K - 1),
                        )
                for m in range(M):
                    y_slice = y_all[:, m, n0 : n0 + 512]
                    nc.vector.tensor_add(y_slice, y_slice, pss[m][:])
                    # incremental RMS stats: ss_m[m] += sum(y_slice^2)
                    sq = sqp.tile([P, 512], F32, tag="sq", name="sq")
                    ssp = statp.tile([P, 1], F32, tag="ssp", name="ssp")
                    nc.scalar.activation(
                        sq[:],
                        y_slice,
                        mybir.ActivationFunctionType.Square,
                        accum_out=ssp[:],
                    )
                    if n == 0:
                        nc.vector.tensor_copy(ss_m[m][:], ssp[:])
                    else:
                        nc.vector.tensor_add(ss_m[m][:], ss_m[m][:], ssp[:])
            for m in range(M):
                y_m = y_all[:, m, :]
                rms = statp.tile([P, 1], F32, tag="rms", name="rms")
                nc.scalar.activation(
                    rms[:],
                    ss_m[m][:],
                    mybir.ActivationFunctionType.Sqrt,
                    bias=eps_sb[:],
                    scale=1.0 / D,
                )
                rstd = statp.tile([P, 1], F32, tag="rstd", name="rstd")
                nc.vector.reciprocal(rstd[:], rms[:])
                yn = ynp.tile([P, D], F32, tag="yn", name="yn")
                nc.scalar.activation(
                    yn[:],
                    y_m,
                    mybir.ActivationFunctionType.Identity,
                    bias=0.0,
                    scale=rstd[:],
                )
                nc.vector.tensor_mul(yn[:], yn[:], nw_sb[:])
                nc.sync.dma_start(Y[m * P : (m + 1) * P, :], yn[:])
    nc.compile()
    return nc


def host_dispatch(xf, Wr, br):
    """Router + top-2 + softmax gates + expert grouping. Returns dispatch info."""
    T, D = xf.shape
    E = Wr.shape[1]
    logits = xf @ Wr + br
    i1 = np.argmax(logits, axis=1)
    l2 = logits.copy()
    l2[np.arange(T), i1] = -np.inf
    i2 = np.argmax(l2, axis=1)
    v1 = logits[np.arange(T), i1]
    v2 = logits[np.arange(T), i2]
    e2 = np.exp(v2 - v1)
    g1 = (1.0 / (1.0 + e2)).astype(np.float32)
    g2 = (e2 / (1.0 + e2)).astype(np.float32)

    # flat slots (t,s) grouped by expert, stable by (token, slot)
    ee = np.stack([i1, i2], 1).ravel()          # [2T]
    gg = np.stack([g1, g2], 1).ravel()
    tt = np.repeat(np.arange(T), 2)
    order = np.argsort(ee, kind="stable")
    counts = np.bincount(ee, minlength=E)
    starts = np.concatenate([[0], np.cumsum(counts)[:-1]])
    rank = np.empty(2 * T, np.int64)
    rank[order] = np.arange(2 * T)
    pos = rank - starts[ee]                      # position within expert's list
    return dict(
        e1=i1, e2=i2, counts=counts, order=order, starts=starts,
        pos=pos.reshape(T, 2), tok=tt, gate=gg, g1=g1, g2=g2,
    )


def bcap_for(counts):
    return int(np.ceil(max(int(counts.max()), 512) / 128) * 128)


def prep_l1_inputs(xf, d, We, be):
    """Per-expert L1 inputs: gathered+pretiled bf16 XT, bf16 W, fp32 be."""
    T, D = xf.shape
    E = We.shape[0]
    K = D // P
    F = D // P
    counts = d["counts"]
    Bcap = bcap_for(counts)
    C = len(_chunks(Bcap, 512))
    Bpad = C * 512
    We_f = np.asarray(We, np.float32)
    be_f = np.asarray(be, np.float32)
    in1 = []
    for e in range(E):
        sel = d["order"][d["starts"][e] : d["starts"][e] + counts[e]]
        Xg = np.zeros((Bpad, D), np.float32)
        Xg[: counts[e]] = xf[d["tok"][sel]]
        # [C, P, K, 512]: contiguous 16KB per (chunk, partition) DMA runs
        XT_T = np.ascontiguousarray(
            Xg.reshape(C, 512, K, P).transpose(0, 3, 2, 1)
        ).astype(BF16)
        W_T = np.ascontiguousarray(
            We_f[e].reshape(K, P, F, P).transpose(2, 1, 0, 3)
        ).astype(BF16)
        in1.append({"XT": XT_T, "W": W_T, "BE": be_f[e]})
    return in1, Bcap


def prep_l2_inputs(xf, d, H, Wo, bo, norm_w):
    """Per-core L2 inputs. CT = g1*A + g2*B combined on host (fp32 math,
    one bf16 downcast); XIN = x + bo in fp32."""
    T, D = xf.shape
    TPC = T // NCORE
    Wo_b = np.asarray(Wo, np.float32).astype(BF16)
    bo_f = np.asarray(bo, np.float32)
    nw_f = np.asarray(norm_w, np.float32)
    e1, e2, pos = d["e1"], d["e2"], d["pos"]
    g1, g2 = d["g1"], d["g2"]
    in2 = []
    for c in range(NCORE):
        tl = np.arange(c * TPC, (c + 1) * TPC)
        CTf = np.empty((D, TPC), np.float32)
        BTf = np.empty((D, TPC), np.float32)
        for e in range(H.shape[0]):
            s1 = e1[tl] == e
            if s1.any():
                CTf[:, s1] = H[e][:, pos[tl[s1], 0]]
            s2 = e2[tl] == e
            if s2.any():
                BTf[:, s2] = H[e][:, pos[tl[s2], 1]]
        CTf = CTf * g1[tl][None, :] + BTf * g2[tl][None, :]
        XIN = xf[tl] + bo_f[None, :]
        in2.append({"CT": CTf.astype(BF16), "XIN": XIN, "WO": Wo_b, "NW": nw_f})
    return in2


# ----------------------------------------------------------------------------
# Harness entry point: full (unsharded) inputs -> full output.
# ----------------------------------------------------------------------------
_L1_CACHE = {}
_L2_CACHE = {}


def kernel(x, Wr, br, We, be, Wo, bo, norm_w):
    B, S, D = x.shape
    E = We.shape[0]
    T = B * S
    TPC = T // NCORE
    xf = np.ascontiguousarray(np.asarray(x, np.float32).reshape(T, D))
    d = host_dispatch(xf, np.asarray(Wr, np.float32), np.asarray(br, np.float32))

    in1, Bcap = prep_l1_inputs(xf, d, We, be)
    if (D, Bcap) not in _L1_CACHE:
        _L1_CACHE[(D, Bcap)] = build_l1(D, Bcap)
    r1 = run_bass_kernel_spmd(_L1_CACHE[(D, Bcap)], in1, list(range(NCORE)))
    H = np.stack([r1.results[e]["H"] for e in range(E)])

    in2 = prep_l2_inputs(xf, d, H, Wo, bo, norm_w)
    if (D, TPC) not in _L2_CACHE:
        _L2_CACHE[(D, TPC)] = build_l2(D, TPC)
    r2 = run_bass_kernel_spmd(_L2_CACHE[(D, TPC)], in2, list(range(NCORE)))
    Y = np.concatenate([r2.results[c]["Y"] for c in range(NCORE)], axis=0)
    return Y.reshape(B, S, D).astype(np.asarray(x).dtype)


# revision 7
# speedup vs baseline: 1.3166x; 1.1040x over previous
# MoE EnhancedGatedFusion kernel for 8x TRN2 NeuronCores (expert-parallel).
#
# Decomposition:
#   host : router logits -> top2 -> softmax gates -> dispatch by expert
#   L1   : per-core (expert e): H_T[d_out, n] = silu(We[e].T-contract @ XT + be[e])
#          (ungated; bf16 matmul operands, fp32 PSUM + fp32 H output)
#   host : combine CT = g1*A + g2*B during the token un-shuffle (the
#          "all-to-all return" glue), downcast once to bf16.
#   L2   : per-core (1024 tokens): out = CT.T @ Wo; y = XIN + out (XIN =
#          x_shard + bo folded on host, fp32); RMS-norm * norm_w.
#
# Matmul operands are bf16: same 1 cycle/row PE rate as fp32r but half the
# HBM/SBUF traffic, and FWL (fast weight load) kicks in for non-fp32 dtypes
# so LDWEIGHTS hides under the 64-deep PE reorder window.
import sys
import types

sys.path.insert(0, "/opt/trn_rl_repo")

import numpy as np
import ml_dtypes

BF16 = np.dtype(ml_dtypes.bfloat16)


def _install_ntff_hook():
    # antenv.axon_hooks is missing in this image; shim it so
    # run_bass_kernel_spmd(trace=True) can drive NTFF profiling.
    if "antenv.axon_hooks" in sys.modules:
        return
    try:
        from trn_agent_boot.trn_boot import _ntff_profile_via_ctypes

        hook = _ntff_profile_via_ctypes("/opt/axon/libaxon_pjrt.so")
    except Exception:
        hook = None
    mod = types.ModuleType("antenv.axon_hooks")
    mod.get_axon_ntff_profile_hook = lambda: hook
    mod.set_axon_ntff_profile_hook = lambda h: None
    sys.modules["antenv.axon_hooks"] = mod


_install_ntff_hook()

import concourse.bacc as bacc
import concourse.bass as bass
import concourse.tile as tile
from concourse import mybir
from concourse.bass_utils import run_bass_kernel_spmd

F32 = mybir.dt.float32
BF = mybir.dt.bfloat16
P = 128
NCORE = 8


def _chunks(total, size):
    out = []
    o = 0
    while o < total:
        out.append((o, min(size, total - o)))
        o += size
    return out


def build_l1(D, Bcap):
    """Per-core expert FFN: H[d_out, n] = silu(sum_k W[k,d_out]*XT[k,n] + be[d_out]).

    XT_T is host-pretiled bf16 [C, P, K, 512] (zero-padded ragged tail) and
    W_T is bf16 [F, P, K, P]; W is fully SBUF-resident (8.4MB), XT streams
    through once.  H output is fp32, staged per chunk in SBUF and written
    with a single DMA per chunk.
    """
    K = D // P          # k-tiles
    F = D // P          # feat (d_out) tiles
    chunks = _chunks(Bcap, 512)
    C = len(chunks)
    nc = bacc.Bacc("TRN2", target_bir_lowering=False, debug=False)
    XT = nc.dram_tensor("XT", [C, P, K, 512], BF, kind="ExternalInput")
    W = nc.dram_tensor("W", [F, P, K, P], BF, kind="ExternalInput")
    BE = nc.dram_tensor("BE", [D], F32, kind="ExternalInput")
    H = nc.dram_tensor("H", [D, Bcap], F32, kind="ExternalOutput")

    Hr = H[:, :].rearrange("(fo p) n -> p fo n", p=P)

    with tile.TileContext(nc) as tc:
        with (
            tc.tile_pool(name="consts", bufs=1) as consts,
            tc.tile_pool(name="xt", bufs=2) as xtp,
            tc.tile_pool(name="wf", bufs=1) as wfp,
            tc.tile_pool(name="hout", bufs=4) as hp,
            tc.tile_pool(name="ps", bufs=4, space="PSUM") as psp,
        ):
            # DMA issue order tuned for the lead-in: w0 first, then the
            # first chunk's XT in two halves, then the remaining W tiles
            # (they stream faster than the f-loop consumes them).
            w_tiles = [None] * F
            w_tiles[0] = wfp.tile([P, K, P], BF, tag="wf0", name="wf0")
            nc.sync.dma_start(w_tiles[0][:], W[0])

            be_sb = consts.tile([P, F], F32)
            nc.sync.dma_start(be_sb[:], BE[:].rearrange("(f p) -> p f", p=P))
            for ci, (c0, cn) in enumerate(chunks):
                xt_c = xtp.tile([P, K, 512], BF, tag="xt", name="xt")
                if ci == 0:
                    nc.sync.dma_start(xt_c[:, 0 : K // 2, :cn],
                                      XT[ci, :, 0 : K // 2, :cn])
                    nc.sync.dma_start(xt_c[:, K // 2 : K, :cn],
                                      XT[ci, :, K // 2 : K, :cn])
                    for f in range(1, F):
                        w_f = wfp.tile([P, K, P], BF, tag=f"wf{f}", name=f"wf{f}")
                        nc.sync.dma_start(w_f[:], W[f])
                        w_tiles[f] = w_f
                else:
                    nc.sync.dma_start(xt_c[:, :, :cn], XT[ci, :, :, :cn])
                for f in range(F):
                    ps = psp.tile([P, 512], F32, tag="ps", name="ps")
                    for k in range(K):
                        nc.tensor.matmul(
                            ps[:, :cn],
                            lhsT=w_tiles[f][:, k, :],
                            rhs=xt_c[:, k, :cn],
                            start=(k == 0),
                            stop=(k == K - 1),
                        )
                    h_t = hp.tile([P, 512], F32, tag="h", name="h")
                    nc.scalar.activation(
                        h_t[:, :cn],
                        ps[:, :cn],
                        mybir.ActivationFunctionType.Silu,
                        bias=be_sb[:, f : f + 1],
                        scale=1.0,
                    )
                    nc.sync.dma_start(Hr[:, f, c0 : c0 + cn], h_t[:, :cn])
    nc.compile()
    return nc


def build_l2(D, TPC, eps=1e-6):
    """Per-core combine + output proj + residual + RMS norm.

    Y[t, j] = nw[j] * (XIN[t,j] + sum_k CT[k,t]*Wo[k,j]) / rms(t)
    CT = g1*A + g2*B (host-combined, bf16); XIN = x_shard + bo (fp32).
    Y output is bf16 (host upcasts).

    The last n-chunk runs m-outer so each m's epilogue (RMS + scale +
    store) chains behind its own k-loop and overlaps the next m's
    matmuls; only m=M-1's epilogue trails the final matmul.
    """
    K = D // P
    M = TPC // P
    NC4 = D // 512
    KB = K // 4          # k-tiles bundled per DMA
    nc = bacc.Bacc("TRN2", target_bir_lowering=False, debug=False)
    CT = nc.dram_tensor("CT", [D, TPC], BF, kind="ExternalInput")
    XIN = nc.dram_tensor("XIN", [TPC, D], F32, kind="ExternalInput")
    WO = nc.dram_tensor("WO", [D, D], BF, kind="ExternalInput")
    NW = nc.dram_tensor("NW", [D], F32, kind="ExternalInput")
    Y = nc.dram_tensor("Y", [TPC, D], BF, kind="ExternalOutput")

    CTr = CT[:, :].rearrange("(kb kk p) n -> p kb kk n", p=P, kk=4)
    WOr = WO[:, :].rearrange("(kb kk p) f -> p kb kk f", p=P, kk=4)
    XINr = XIN[:, :].rearrange("(m p) d -> p m d", p=P)

    with tile.TileContext(nc) as tc:
        with (
            tc.tile_pool(name="consts", bufs=1) as consts,
            tc.tile_pool(name="ct", bufs=1) as ctp,
            tc.tile_pool(name="wo", bufs=2) as wop,
            tc.tile_pool(name="yall", bufs=1) as yallp,
            tc.tile_pool(name="sq", bufs=3) as sqp,
            tc.tile_pool(name="yn", bufs=2) as ynp,
            tc.tile_pool(name="ssm", bufs=1) as ssmp,
            tc.tile_pool(name="stat", bufs=4) as statp,
            tc.tile_pool(name="ps", bufs=1, space="PSUM") as psp,
        ):
            # Bundled DMAs (4 k-tiles each): few descriptors, so the sync
            # engine's issue rate never gates the lead-in.
            ct_bs = []
            for b in range(KB):
                c_b = ctp.tile([P, 4, TPC], BF, tag=f"ct{b}", name=f"ct{b}")
                nc.sync.dma_start(c_b[:], CTr[:, b, :, :])
                ct_bs.append(c_b)
            wo_cur = []
            for b in range(KB):
                w_b = wop.tile([P, 4, 512], BF, tag=f"wo{b}", name=f"wo{b}")
                nc.sync.dma_start(w_b[:], WOr[:, b, :, 0:512])
                wo_cur.append(w_b)
            # y_all preloaded with the residual input (XIN = x + bo): psum
            # evictions then just add into it.  Per-m loads so the first
            # eviction doesn't wait for the full 8.4MB.
            y_all = yallp.tile([P, M, D], F32)
            for m in range(M):
                nc.sync.dma_start(y_all[:, m, :], XINr[:, m, :])
            nw_sb = consts.tile([P, D], F32)
            nwap = NW[:]
            nw_bcast = bass.AP(
                tensor=nwap.tensor, offset=nwap.offset, ap=[[0, P]] + list(nwap.ap)
            )
            nc.sync.dma_start(nw_sb[:], nw_bcast)
            eps_sb = consts.tile([P, 1], F32)
            nc.vector.memset(eps_sb[:], eps)

            ss_m = [ssmp.tile([P, 1], F32, tag=f"ssm{m}", name=f"ssm{m}")
                    for m in range(M)]

            def stats(m, n, y_slice):
                # incremental RMS stats: ss_m[m] += sum(y_slice^2)
                sq = sqp.tile([P, 512], F32, tag="sq", name="sq")
                ssp = statp.tile([P, 1], F32, tag="ssp", name="ssp")
                nc.scalar.activation(
                    sq[:],
                    y_slice,
                    mybir.ActivationFunctionType.Square,
                    accum_out=ssp[:],
                )
                if n == 0:
                    nc.vector.tensor_copy(ss_m[m][:], ssp[:])
                else:
                    nc.vector.tensor_add(ss_m[m][:], ss_m[m][:], ssp[:])

            def epilogue(m):
                y_m = y_all[:, m, :]
                rms = statp.tile([P, 1], F32, tag="rms", name="rms")
                nc.scalar.activation(
                    rms[:],
                    ss_m[m][:],
                    mybir.ActivationFunctionType.Sqrt,
                    bias=eps_sb[:],
                    scale=1.0 / D,
                )
                rstd = statp.tile([P, 1], F32, tag="rstd", name="rstd")
                nc.vector.reciprocal(rstd[:], rms[:])
                yn = ynp.tile([P, D], BF, tag="yn", name="yn")
                nc.vector.scalar_tensor_tensor(
                    yn[:],
                    y_m,
                    rstd[:],
                    nw_sb[:],
                    op0=mybir.AluOpType.mult,
                    op1=mybir.AluOpType.mult,
                )
                nc.sync.dma_start(Y[m * P : (m + 1) * P, :], yn[:])

            pss = [psp.tile([P, 512], F32, tag=f"ps{m}", name=f"ps{m}")
                   for m in range(M)]
            for n in range(NC4):
                n0 = n * 512
                if n + 1 < NC4:
                    wo_nxt = []
                    for b in range(KB):
                        w_b = wop.tile([P, 4, 512], BF, tag=f"wo{b}", name=f"wo{b}")
                        nc.sync.dma_start(w_b[:], WOr[:, b, :, n0 + 512 : n0 + 1024])
                        wo_nxt.append(w_b)
                if n + 1 < NC4:
                    # k-outer: rhs (wo) stays hot across the m sweep
                    for k in range(K):
                        for m in range(M):
                            nc.tensor.matmul(
                                pss[m][:],
                                lhsT=ct_bs[k // 4][:, k % 4, m * P : (m + 1) * P],
                                rhs=wo_cur[k // 4][:, k % 4, :],
                                start=(k == 0),
                                stop=(k == K - 1),
                            )
                    for m in range(M):
                        y_slice = y_all[:, m, n0 : n0 + 512]
                        nc.vector.tensor_add(y_slice, y_slice, pss[m][:])
                        stats(m, n, y_slice)
                    wo_cur = wo_nxt
                else:
                    # last n-chunk: m-outer with chained per-m epilogues
                    for m in range(M):
                        for k in range(K):
                            nc.tensor.matmul(
                                pss[m][:],
                                lhsT=ct_bs[k // 4][:, k % 4, m * P : (m + 1) * P],
                                rhs=wo_cur[k // 4][:, k % 4, :],
                                start=(k == 0),
                                stop=(k == K - 1),
                            )
                        y_slice = y_all[:, m, n0 : n0 + 512]
                        nc.vector.tensor_add(y_slice, y_slice, pss[m][:])
                        stats(m, n, y_slice)
                        epilogue(m)
    nc.compile()
    return nc


def host_dispatch(xf, Wr, br):
    """Router + top-2 + softmax gates + expert grouping. Returns dispatch info."""
    T, D = xf.shape
    E = Wr.shape[1]
    logits = xf @ Wr + br
    i1 = np.argmax(logits, axis=1)
    l2 = logits.copy()
    l2[np.arange(T), i1] = -np.inf
    i2 = np.argmax(l2, axis=1)
    v1 = logits[np.arange(T), i1]
    v2 = logits[np.arange(T), i2]
    e2 = np.exp(v2 - v1)
    g1 = (1.0 / (1.0 + e2)).astype(np.float32)
    g2 = (e2 / (1.0 + e2)).astype(np.float32)

    # flat slots (t,s) grouped by expert, stable by (token, slot)
    ee = np.stack([i1, i2], 1).ravel()          # [2T]
    gg = np.stack([g1, g2], 1).ravel()
    tt = np.repeat(np.arange(T), 2)
    order = np.argsort(ee, kind="stable")
    counts = np.bincount(ee, minlength=E)
    starts = np.concatenate([[0], np.cumsum(counts)[:-1]])
    rank = np.empty(2 * T, np.int64)
    rank[order] = np.arange(2 * T)
    pos = rank - starts[ee]                      # position within expert's list
    return dict(
        e1=i1, e2=i2, counts=counts, order=order, starts=starts,
        pos=pos.reshape(T, 2), tok=tt, gate=gg, g1=g1, g2=g2,
    )


def bcap_for(counts):
    return int(np.ceil(max(int(counts.max()), 512) / 128) * 128)


def prep_l1_inputs(xf, d, We, be):
    """Per-expert L1 inputs: gathered+pretiled bf16 XT, bf16 W, fp32 be."""
    T, D = xf.shape
    E = We.shape[0]
    K = D // P
    F = D // P
    counts = d["counts"]
    Bcap = bcap_for(counts)
    C = len(_chunks(Bcap, 512))
    Bpad = C * 512
    We_f = np.asarray(We, np.float32)
    be_f = np.asarray(be, np.float32)
    in1 = []
    for e in range(E):
        sel = d["order"][d["starts"][e] : d["starts"][e] + counts[e]]
        Xg = np.zeros((Bpad, D), np.float32)
        Xg[: counts[e]] = xf[d["tok"][sel]]
        # [C, P, K, 512]: contiguous 16KB per (chunk, partition) DMA runs
        XT_T = np.ascontiguousarray(
            Xg.reshape(C, 512, K, P).transpose(0, 3, 2, 1)
        ).astype(BF16)
        W_T = np.ascontiguousarray(
            We_f[e].reshape(K, P, F, P).transpose(2, 1, 0, 3)
        ).astype(BF16)
        in1.append({"XT": XT_T, "W": W_T, "BE": be_f[e]})
    return in1, Bcap


def prep_l2_inputs(xf, d, H, Wo, bo, norm_w):
    """Per-core L2 inputs. CT = g1*A + g2*B combined on host (fp32 math,
    one bf16 downcast); XIN = x + bo in fp32."""
    T, D = xf.shape
    TPC = T // NCORE
    Wo_b = np.asarray(Wo, np.float32).astype(BF16)
    bo_f = np.asarray(bo, np.float32)
    nw_f = np.asarray(norm_w, np.float32)
    e1, e2, pos = d["e1"], d["e2"], d["pos"]
    g1, g2 = d["g1"], d["g2"]
    in2 = []
    for c in range(NCORE):
        tl = np.arange(c * TPC, (c + 1) * TPC)
        CTf = np.empty((D, TPC), np.float32)
        BTf = np.empty((D, TPC), np.float32)
        for e in range(H.shape[0]):
            s1 = e1[tl] == e
            if s1.any():
                CTf[:, s1] = H[e][:, pos[tl[s1], 0]]
            s2 = e2[tl] == e
            if s2.any():
                BTf[:, s2] = H[e][:, pos[tl[s2], 1]]
        CTf = CTf * g1[tl][None, :] + BTf * g2[tl][None, :]
        XIN = xf[tl] + bo_f[None, :]
        in2.append({"CT": CTf.astype(BF16), "XIN": XIN, "WO": Wo_b, "NW": nw_f})
    return in2


# ----------------------------------------------------------------------------
# Harness entry point: full (unsharded) inputs -> full output.
# ----------------------------------------------------------------------------
_L1_CACHE = {}
_L2_CACHE = {}


def kernel(x, Wr, br, We, be, Wo, bo, norm_w):
    B, S, D = x.shape
    E = We.shape[0]
    T = B * S
    TPC = T // NCORE
    xf = np.ascontiguousarray(np.asarray(x, np.float32).reshape(T, D))
    d = host_dispatch(xf, np.asarray(Wr, np.float32), np.asarray(br, np.float32))

    in1, Bcap = prep_l1_inputs(xf, d, We, be)
    if (D, Bcap) not in _L1_CACHE:
        _L1_CACHE[(D, Bcap)] = build_l1(D, Bcap)
    r1 = run_bass_kernel_spmd(_L1_CACHE[(D, Bcap)], in1, list(range(NCORE)))
    H = np.stack([r1.results[e]["H"] for e in range(E)])

    in2 = prep_l2_inputs(xf, d, H, Wo, bo, norm_w)
    if (D, TPC) not in _L2_CACHE:
        _L2_CACHE[(D, TPC)] = build_l2(D, TPC)
    r2 = run_bass_kernel_spmd(_L2_CACHE[(D, TPC)], in2, list(range(NCORE)))
    Y = np.concatenate([r2.results[c]["Y"] for c in range(NCORE)], axis=0)
    return Y.reshape(B, S, D).astype(np.asarray(x).dtype)
